# revision 8
# baseline (speedup 1.0000x reference)
"""Distributed Trainium2 Bass kernel for the GAT-style attention layer.

Reference computation (N=8192, D_IN=512, D_OUT=256):
    h = x @ W.T                       [N, D_OUT]
    f1 = h @ a1; f2 = h @ a2          [N]
    e = leaky_relu(f1[:,None] + f2[None,:], 0.01) * adj
    e = where(e == 0, -1e9, e)
    alpha = softmax(e, axis=1)
    out = elu(alpha @ h)              [N, D_OUT]

Distribution: row-parallel over nodes across 8 NeuronCores. Each core owns
ROWS = N/8 rows of x / e / out; W, a1, a2 are replicated; h (plus f2) is
all-gathered so每 core computes its row block of scores, softmax and
aggregation locally.

Key device-side algebra (all FLOPs happen on device):
  - p_ij = exp(leaky_relu(s_ij)) with s = f1_i + f2_j is computed as
        p = max(exp(f1_i) * exp(f2_j), 1 + 0.01 * s)
    The first branch is exact for s > 0 (exp factorizes over the rank-2
    score matrix, making it a cheap DVE outer product); the second branch
    approximates exp(0.01 s) to < 1e-2 relative over the attainable range
    of s (|s| < ~15), and the max() selects exactly the right branch.
  - masking multiplies by adj in {0,1}; exp(-1e9) == 0 is reproduced by
    p * adj since p > 0.
  - softmax denominator comes for free as an extra all-ones column of the
    gathered h (the big matmul computes [alpha_unnormalized @ (h | 1)]).
  - elu(y) = min(exp(y) - 1, relu(y)).

The score matrix is produced directly in transposed [j, i] layout (so the
P^T @ h matmul needs no on-chip transposes); adj arrives transposed via the
DMA xbar transpose (bf16) straight from HBM.
"""

import numpy as np

import concourse.bass as bass
import concourse.mybir as mybir
from concourse.tile import TileContext
from concourse.vector_clock import ScopedClock
from concourse.bass_utils import run_bass_kernel_spmd

# ----------------------------------------------------------------------------
# Problem constants (hardcoded per the harness contract)
N = 8192
D_IN = 512
D_OUT = 256
N_CORES = 8
ROWS = N // N_CORES          # 1024 rows per core
P = 128                      # SBUF partitions

AluOp = mybir.AluOpType
Act = mybir.ActivationFunctionType
F32 = mybir.dt.float32
BF16 = mybir.dt.bfloat16


# ----------------------------------------------------------------------------
# The walrus build in this toolchain accepts only ONE sync-wait condition per
# instruction (setupSyncWait "Too many sync wait commands"). Tile's scheduler
# can emit several waits on one instruction (e.g. a matmul waiting on both of
# its input DMAs, or the tail drain waiting on every live semaphore). Post-
# process the finished module: move excess waits onto same-engine NOPs placed
# immediately before the instruction — the engine's NX dispatches in order, so
# stalling on the NOPs first is equivalent.
def _split_excess_waits(nc, max_waits=1):
    n_split = [0]

    def fix_block(b):
        new_insts = []
        for inst in b.instructions:
            si = getattr(inst, "sync_info", None)
            if si is not None and si.on_wait and len(si.on_wait) > max_waits:
                waits = list(si.on_wait)
                extra, keep = waits[:-max_waits], waits[-max_waits:]
                for w in extra:
                    n_split[0] += 1
                    nop = mybir.InstEventSemaphore(
                        name=f"waitsplit-{n_split[0]}", ins=[], outs=[]
                    )
                    nop.engine = inst.engine
                    nop.sync_info = mybir.SyncInfo(on_wait=[w], on_update=[])
                    new_insts.append(nop)
                inst.sync_info = mybir.SyncInfo(
                    on_wait=keep, on_update=list(si.on_update or [])
                )
            new_insts.append(inst)
        b.instructions[:] = new_insts

    for f in nc.m.functions:
        for b in f.blocks:
            fix_block(b)
    return n_split[0]


# ----------------------------------------------------------------------------
def build_nc(
    n_cores: int = N_CORES,
    rows: int = ROWS,
    n: int = N,
    d_in: int = D_IN,
    d_out: int = D_OUT,
    cb: int = 16,              # j-chunks per adj transpose DMA block
    gpsimd_mask_mod: int = 0,  # if k>0: every k-th chunk's mask-mult on GpSimd
    split_waits: bool = True,  # walrus workaround; disable for CoreSim runs
):
    """Build the SPMD graph executed identically on every core."""

    n_it = rows // P           # i-tiles per core (8)
    n_kc = d_in // P           # contraction chunks for the h matmul (4)
    n_jc = n // P              # total j-chunks (64)
    nb = n_jc // cb            # adj transpose blocks
    assert n_jc % cb == 0
    dh = d_out + 1             # h | ones

    nc = bass.Bass(num_devices=n_cores)

    xT = nc.declare_dram_parameter("xT", [d_in, rows], F32, isOutput=False)
    wT = nc.declare_dram_parameter("wT", [d_in, d_out], F32, isOutput=False)
    a12 = nc.declare_dram_parameter("a12", [2, d_out], F32, isOutput=False)
    adjb = nc.declare_dram_parameter("adjb", [rows, n], BF16, isOutput=False)
    out_ext = nc.declare_dram_parameter("out", [rows, d_out], F32, isOutput=True)

    rg = [list(range(n_cores))]

    with TileContext(nc) as tc:
        from contextlib import ExitStack

        with ExitStack() as ctx:
            # ---------------- constant / resident tiles
            const = ctx.enter_context(tc.tile_pool(name="const", bufs=1))
            f1b = const.tile([P, rows], BF16)      # f1 along free dim, bcast over partitions
            ef1b = const.tile([P, rows], BF16)     # exp(f1) likewise
            f2sb = const.tile([P, n_jc], F32)      # f2 column-major: [p, c] = f2[c*128+p]
            ef2sb = const.tile([P, n_jc], F32)
            f2c01 = const.tile([P, n_jc], F32)     # 1 + 0.01*f2

            # ---------------- DRAM bounce tiles (tracked by Tile)
            dram = ctx.enter_context(tc.tile_pool(name="dram", bufs=1, space="DRAM"))
            hloc = dram.tile([rows, dh], BF16)
            f1d = dram.tile([rows], F32)
            f2loc = dram.tile([rows], F32)
            hfull = dram.tile([n, dh], BF16, addr_space="Shared")
            f2full = dram.tile([n], F32, addr_space="Shared")

            # ---------------- phase A: h = x @ W.T, f1/f2, gathers
            with tc.tile_pool(name="ph1", bufs=1) as ph1, tc.tile_pool(
                name="ph1ps", bufs=2, space="PSUM"
            ) as ph1ps:
                xt_sb = []
                wt_sb = []
                for k in range(n_kc):
                    xk = ph1.tile([P, rows], F32, name=f"xt{k}")
                    wk = ph1.tile([P, d_out], F32, name=f"wt{k}")
                    nc.sync.dma_start(out=xk[:], in_=xT[k * P : (k + 1) * P, :])
                    nc.sync.dma_start(out=wk[:], in_=wT[k * P : (k + 1) * P, :])
                    xt_sb.append(xk)
                    wt_sb.append(wk)
                # a1/a2 broadcast along partitions: [2, d_out] -> [128, d_out] each
                a1b = ph1.tile([P, d_out], F32, name="a1b")
                a2b = ph1.tile([P, d_out], F32, name="a2b")
                nc.sync.dma_start(out=a1b[:], in_=a12[0:1, :].to_broadcast((P, d_out)))
                nc.sync.dma_start(out=a2b[:], in_=a12[1:2, :].to_broadcast((P, d_out)))

                fcols = ph1.tile([P, 2 * n_it], F32, name="fcols")
                ftmp = ph1.tile([P, d_out], F32, name="ftmp")
                for t in range(n_it):
                    ps = ph1ps.tile([P, d_out], F32, name="psh")
                    for k in range(n_kc):
                        nc.tensor.matmul(
                            ps[:],
                            xt_sb[k][:, t * P : (t + 1) * P],
                            wt_sb[k][:],
                            start=(k == 0),
                            stop=(k == n_kc - 1),
                        )
                    haug = ph1.tile([P, dh], BF16, name="haug", tag="haug", bufs=2)
                    nc.scalar.copy(out=haug[:, 0:d_out], in_=ps[:])
                    nc.vector.memset(haug[:, d_out:dh], 1.0)
                    nc.sync.dma_start(
                        out=hloc[t * P : (t + 1) * P, :], in_=haug[:]
                    )
                    # f1/f2 for this i-tile: multiply then reduce over free dim
                    nc.vector.tensor_tensor(
                        out=ftmp[:], in0=ps[:], in1=a1b[:], op=AluOp.mult
                    )
                    nc.vector.reduce_sum(
                        out=fcols[:, 2 * t : 2 * t + 1],
                        in_=ftmp[:],
                        axis=mybir.AxisListType.X,
                    )
                    nc.vector.tensor_tensor(
                        out=ftmp[:], in0=ps[:], in1=a2b[:], op=AluOp.mult
                    )
                    nc.vector.reduce_sum(
                        out=fcols[:, 2 * t + 1 : 2 * t + 2],
                        in_=ftmp[:],
                        axis=mybir.AxisListType.X,
                    )
                # f1 / f2_local to DRAM ([p, t] layout -> linear [t*128+p])
                nc.sync.dma_start(
                    out=f1d[:].rearrange("(t p) -> p t", p=P),
                    in_=fcols[:, 0 : 2 * n_it : 2],
                )
                nc.sync.dma_start(
                    out=f2loc[:].rearrange("(t p) -> p t", p=P),
                    in_=fcols[:, 1 : 2 * n_it : 2],
                )
                nc.gpsimd.collective_compute(
                    "AllGather",
                    AluOp.bypass,
                    replica_groups=rg,
                    ins=[f2loc[:]],
                    outs=[f2full[:]],
                )
                nc.gpsimd.collective_compute(
                    "AllGather",
                    AluOp.bypass,
                    replica_groups=rg,
                    ins=[hloc[:]],
                    outs=[hfull[:]],
                )
                # broadcast f1 back across partitions; build resident tiles
                f1b32 = ph1.tile([P, rows], F32, name="f1b32")
                nc.sync.dma_start(
                    out=f1b32[:], in_=f1d[:][None, :].to_broadcast((P, rows))
                )
                nc.vector.tensor_copy(out=f1b[:], in_=f1b32[:])
                nc.scalar.activation(out=ef1b[:], in_=f1b32[:], func=Act.Exp)
                nc.sync.dma_start(
                    out=f2sb[:], in_=f2full[:].rearrange("(c p) -> p c", p=P)
                )
                nc.scalar.activation(out=ef2sb[:], in_=f2sb[:], func=Act.Exp)
                nc.vector.tensor_scalar(
                    out=f2c01[:],
                    in0=f2sb[:],
                    scalar1=0.01,
                    scalar2=1.0,
                    op0=AluOp.mult,
                    op1=AluOp.add,
                )

            # ---------------- phase B: scores + mask + matmul over j-chunks
            mainps = ctx.enter_context(
                tc.tile_pool(name="mainps", bufs=1, space="PSUM")
            )
            psum_o = [mainps.tile([P, dh], F32, name=f"pso{t}") for t in range(n_it)]

            adj_pool = ctx.enter_context(tc.tile_pool(name="adjp", bufs=2))
            l_pool = ctx.enter_context(tc.tile_pool(name="lp", bufs=3))
            p_pool = ctx.enter_context(tc.tile_pool(name="pp", bufs=3))
            h_pool = ctx.enter_context(tc.tile_pool(name="hp", bufs=3))

            for b in range(nb):
                adjT = adj_pool.tile([P, cb * rows], BF16, name="adjT", tag="adjT")
                nc.sync.dma_start_transpose(
                    out=adjT[:].rearrange("p (c f) -> p c f", f=rows),
                    in_=adjb[:, b * cb * P : (b + 1) * cb * P],
                )
                for ci in range(cb):
                    c = b * cb + ci
                    # L = 1 + 0.01*(f1_i + f2_j)  (linear branch)
                    lw = l_pool.tile([P, rows], BF16, name="lw", tag="lw")
                    nc.vector.tensor_scalar(
                        out=lw[:],
                        in0=f1b[:],
                        scalar1=0.01,
                        scalar2=f2c01[:, c : c + 1],
                        op0=AluOp.mult,
                        op1=AluOp.add,
                    )
                    # M = max(exp(f1)*exp(f2), L)
                    mw = p_pool.tile([P, rows], BF16, name="mw", tag="mw")
                    nc.vector.scalar_tensor_tensor(
                        out=mw[:],
                        in0=ef1b[:],
                        scalar=ef2sb[:, c : c + 1],
                        in1=lw[:],
                        op0=AluOp.mult,
                        op1=AluOp.max,
                    )
                    # mask: P^T = M * adjT
                    if gpsimd_mask_mod and (c % gpsimd_mask_mod == 0):
                        eng = nc.gpsimd
                    else:
                        eng = nc.vector
                    eng.tensor_tensor(
                        out=mw[:],
                        in0=mw[:],
                        in1=adjT[:, ci * rows : (ci + 1) * rows],
                        op=AluOp.mult,
                    )
                    # gathered h chunk (with ones column)
                    hc = h_pool.tile([P, dh], BF16, name="hc", tag="hc")
                    nc.sync.dma_start(out=hc[:], in_=hfull[c * P : (c + 1) * P, :])
                    for t in range(n_it):
                        nc.tensor.matmul(
                            psum_o[t][:],
                            mw[:, t * P : (t + 1) * P],
                            hc[:],
                            start=(c == 0),
                            stop=(c == n_jc - 1),
                        )

            # ---------------- epilogue: normalize + elu + store
            ep = ctx.enter_context(tc.tile_pool(name="ep", bufs=1))
            y = ep.tile([P, n_it * d_out], F32)
            e = ep.tile([P, n_it * d_out], F32)
            rc = ep.tile([P, n_it], F32)
            for t in range(n_it):
                nc.vector.reciprocal(out=rc[:, t : t + 1], in_=psum_o[t][:, d_out:dh])
                nc.scalar.mul(
                    out=y[:, t * d_out : (t + 1) * d_out],
                    in_=psum_o[t][:, 0:d_out],
                    mul=rc[:, t : t + 1],
                )
            nc.scalar.activation(out=e[:], in_=y[:], func=Act.Exp)
            nc.vector.tensor_scalar(
                out=e[:], in0=e[:], scalar1=1.0, scalar2=None, op0=AluOp.subtract
            )
            nc.scalar.activation(out=y[:], in_=y[:], func=Act.Relu)
            nc.vector.tensor_tensor(out=e[:], in0=e[:], in1=y[:], op=AluOp.min)
            nc.sync.dma_start(
                out=out_ext[:].rearrange("(t p) d -> p t d", p=P),
                in_=e[:].rearrange("p (t d) -> p t d", d=d_out),
            )

    if split_waits:
        _split_excess_waits(nc)
    return nc


# ----------------------------------------------------------------------------
def make_in_maps(x, adj_mat, W, a1, a2, n_cores=N_CORES):
    """Shard + lay out the full inputs for each core. Layout/dtype prep only."""
    import ml_dtypes

    rows = x.shape[0] // n_cores
    wT = np.ascontiguousarray(W.T, dtype=np.float32)            # [d_in, d_out]
    a12 = np.ascontiguousarray(
        np.stack([a1[:, 0], a2[:, 0]], axis=0), dtype=np.float32
    )                                                            # [2, d_out]
    in_maps = []
    for i in range(n_cores):
        sl = slice(i * rows, (i + 1) * rows)
        in_maps.append(
            {
                "xT": np.ascontiguousarray(x[sl].T, dtype=np.float32),
                "wT": wT,
                "a12": a12,
                "adjb": np.ascontiguousarray(
                    adj_mat[sl].astype(ml_dtypes.bfloat16)
                ),
            }
        )
    return in_maps


_NC_CACHE = {}


def kernel(x, adj_mat, W, a1, a2):
    x = np.asarray(x)
    adj_mat = np.asarray(adj_mat)
    W = np.asarray(W)
    a1 = np.asarray(a1)
    a2 = np.asarray(a2)

    in_maps = make_in_maps(x, adj_mat, W, a1, a2)
    if "nc" not in _NC_CACHE:
        _NC_CACHE["nc"] = build_nc()
    nc = _NC_CACHE["nc"]
    res = run_bass_kernel_spmd(nc, in_maps, list(range(N_CORES)))
    out = np.concatenate([res.results[i]["out"] for i in range(N_CORES)], axis=0)
    return np.ascontiguousarray(out, dtype=np.float32)
